# revision 24
# baseline (speedup 1.0000x reference)
"""Causal multi-head attention layer for Trainium2, SPMD across 8 NeuronCores.

Sharding: batch (B=2) x head-quads (16 heads -> 4 groups of 4) = 8 shards.
Core c handles batch c//4, heads 4*(c%4) .. 4*(c%4)+3.

v2: bf16 end-to-end (host-converted), causal mask folded into the scores
PSUM via an extra accumulating matmul (identity @ mask-triangle), Q/K
projection PSUM->SBUF copies on the vector engine (scalar engine does exp
only), and a software-pipelined schedule: a minimal prologue computes just
the V/Q/K tiles the first attention chunk needs, and every remaining
projection / output-projection matmul is woven one instruction at a time
between attention steps so the tensor engine stays dense while the scalar
engine works through the softmax exps.

Per core:
  - host pre-transposes activations to [d_model, tokens] bf16
  - Q^T,K^T projected as [dk*2, L] pairs (2 heads stacked on partitions),
    V projected in natural [S, dk] layout with a ones column appended
  - scores computed transposed [S, L]; mask as -1e30 matmul into PSUM;
    exp on the scalar engine; E @ [V|1] yields O^T plus softmax row-sums;
    normalization via reciprocal + ones-row matmul broadcast
  - output projection accumulates head pairs; host sums the 4 partial
    projections per batch and adds bo.
"""
import json

import numpy as np

import concourse.bass as bass
import concourse.mybir as mybir
import concourse.tile as tile
from concourse.ap import AP as BassAP

F32 = mybir.dt.float32
F32R = mybir.dt.float32r
BF16 = mybir.dt.bfloat16

D = 1024        # d_model
T = 2048        # tokens (L = S)
HC = 4          # heads per core
CW = 256        # projection cols per core (HC * 64)
KC = 8          # k chunks of 128 over D
NJ = 4          # l-chunks of 512
LCW = 512       # l chunk width
NST = 16        # s tiles of 128
P = 128
DK = 64
LAG = 3
NEG = -1e30
NORM_BCAST = False  # 1/r broadcast: False=PE matmul, True=SWDGE stride-0 DMA


# ---------------------------------------------------------------------------
# walrus in this container allows at most ONE sync-wait command per
# instruction; split extras onto preceding NoOps on the same engine
# (sequencers execute in order, so semantics are identical).
_orig_to_json_bytes = bass.Bass.to_json_bytes
_CTR = [0]


def _legalize(bir):
    for fn in bir.get("functions", []):
        for bb in fn.get("blocks", []):
            insts = bb.get("instructions", [])
            if not any(
                len((i.get("sync_info") or {}).get("on_wait") or []) > 1
                for i in insts
            ):
                continue
            out = []
            for inst in insts:
                si = inst.get("sync_info")
                waits = (si or {}).get("on_wait") or []
                if len(waits) > 1:
                    for w in waits[:-1]:
                        _CTR[0] += 1
                        nop = {
                            "engine": inst["engine"],
                            "ins": [],
                            "outs": [],
                            "name": f"lw-nop-{_CTR[0]}",
                            "opcode": "NoOp",
                            "sync_info": {"on_update": [], "on_wait": [w]},
                        }
                        if "debug" in inst:
                            nop["debug"] = inst["debug"]
                        out.append(nop)
                    si["on_wait"] = [waits[-1]]
                out.append(inst)
            bb["instructions"] = out
    return bir


def _patched_to_json_bytes(self):
    bir = json.loads(_orig_to_json_bytes(self))
    return json.dumps(_legalize(bir)).encode()


def install_legalizer():
    bass.Bass.to_json_bytes = _patched_to_json_bytes


# ---------------------------------------------------------------------------
def build(repeat=1):
    nc = bass.Bass("TRN2", target_bir_lowering=False, debug=False, num_devices=8)
    aps = {}
    for nm, shp, dt in [
        ("xq_t", [D, T], BF16), ("xk_t", [D, T], BF16), ("xv_t", [D, T], BF16),
        ("wq", [P, KC, CW], BF16), ("wk", [P, KC, CW], BF16),
        ("wv", [P, KC, CW], BF16),
        ("wo", [P, 2, D], BF16), ("bq3", [2, P, 1], F32), ("bk3", [2, P, 1], F32),
        ("bv2", [1, 2 * CW], BF16), ("mtri", [P, P], BF16),
        ("ident", [P, P], BF16),
    ]:
        aps[nm] = nc.dram_tensor(nm, shp, dt, kind="ExternalInput").ap()
    aps["out_p"] = nc.dram_tensor("out_p", [T, D], F32, kind="ExternalOutput").ap()

    with tile.TileContext(nc) as tc:
        for _ in range(repeat):
            _body(tc, nc, aps)
    return nc


def _body(tc, nc, aps):
    from contextlib import ExitStack
    ctx = ExitStack()
    with ctx:
        ctx.enter_context(nc.allow_low_precision(
            reason="bf16 compute is intentional; fp32 accumulate in PSUM"))
        singles = ctx.enter_context(tc.tile_pool(name="singles", bufs=1))
        xt_pool = ctx.enter_context(tc.tile_pool(name="xt", bufs=12))
        et_pool = ctx.enter_context(tc.tile_pool(name="et", bufs=4))
        recip_pool = ctx.enter_context(tc.tile_pool(name="recip", bufs=2))
        rbsb_pool = ctx.enter_context(tc.tile_pool(name="rbsb", bufs=2))
        osb_pool = ctx.enter_context(tc.tile_pool(name="osb", bufs=3))
        # PSUM budget (8 banks): scores 5x1 (per-e tiles, deep rotation to
        # hide the PE<->ACT sem round-trip) + attn-out 2x1 + misc 1x1
        ps_s_pool = ctx.enter_context(tc.tile_pool(name="pss", bufs=5, space="PSUM"))
        ps_o_pool = ctx.enter_context(tc.tile_pool(name="pso", bufs=2, space="PSUM"))
        misc_ps = ctx.enter_context(tc.tile_pool(name="mps", bufs=1, space="PSUM"))

        wq_sb = singles.tile([P, KC, CW], BF16, tag="wq")
        wk_sb = singles.tile([P, KC, CW], BF16, tag="wk")
        wv_sb = singles.tile([P, KC, CW], BF16, tag="wv")
        wo_sb = singles.tile([P, 2, D], BF16, tag="wo")
        bq_sb = [singles.tile([P, 1], F32, tag=f"bq{p}", name=f"bq_sb{p}") for p in range(2)]
        bk_sb = [singles.tile([P, 1], F32, tag=f"bk{p}", name=f"bk_sb{p}") for p in range(2)]
        bv_sb = singles.tile([1, 2 * CW], BF16, tag="bv")
        mtri_sb = singles.tile([P, P], BF16, tag="mtri")
        ident_sb = singles.tile([P, P], BF16, tag="ident")
        ones_sb = singles.tile([1, P], BF16, tag="ones")
        nc.vector.memset(ones_sb, 1.0)
        onesatt = singles.tile([DK + 1, DK], F32R, tag="onesatt")
        ones_f32 = singles.tile([DK + 1, DK], F32, tag="ones_f32")
        nc.vector.memset(ones_f32[DK:DK + 1, :], 1.0)
        nc.vector.tensor_copy(onesatt[DK:DK + 1, :], ones_f32[DK:DK + 1, :])

        qt_sb = [singles.tile([P, T], BF16, tag=f"qt{p}", name=f"qt_sb{p}") for p in range(2)]
        kt_sb = [singles.tile([P, T], BF16, tag=f"kt{p}", name=f"kt_sb{p}") for p in range(2)]
        ot_sb = [singles.tile([P, T], BF16, tag=f"ot{p}", name=f"ot_sb{p}") for p in range(2)]
        v_sb = [singles.tile([P, HC, DK + 1], BF16, tag=f"v{i}", name=f"v_sb{i}")
                for i in range(NST)]

        # ===== staging: one [P, KC, 512] tile + one DMA per (input, quarter)
        xq_st = [None] * 4
        xk_st = [None] * 4
        xv_st = [None] * 4

        def stage_q(store, x_ap, q, tagpfx, engine, split=False):
            xt = xt_pool.tile([P, KC, LCW], BF16, tag="xt",
                              name=f"{tagpfx}{q}")
            src = x_ap.rearrange("(ko ki) t -> ki ko t", ki=P)[
                :, :, q * LCW:(q + 1) * LCW]
            if split:
                # per-k DMAs into one tile: fine-grained completion lets the
                # prologue matmuls start as soon as their k-chunk lands
                for k in range(KC):
                    engine.dma_start(out=xt[:, k, :], in_=src[:, k, :])
            else:
                engine.dma_start(out=xt, in_=src)
            store[q] = xt

        # ===== generator work units (each yield = one PE matmul emitted) ===
        def qk_unit(w_sb, b_sb, dst, p, lc, xst):
            ps = misc_ps.tile([P, LCW], F32, tag="m", name=f"qk{p}{lc}")
            for k in range(KC):
                nc.tensor.matmul(
                    ps, w_sb[:, k, p * P:(p + 1) * P],
                    xst[lc][:, k, :],
                    start=(k == 0), stop=(k == KC - 1))
                yield
            nc.vector.tensor_scalar_add(
                dst[p][:, lc * LCW:(lc + 1) * LCW], ps, b_sb[p][:, 0:1])

        def v_unit(st2, xst):
            ps = misc_ps.tile([P, 2 * CW], F32, tag="m", name=f"v{st2}")
            q, qsub = divmod(st2, 2)
            for k in range(KC):
                for sub in range(2):
                    il = qsub * 2 + sub
                    nc.tensor.matmul(
                        ps[:, sub * CW:(sub + 1) * CW],
                        xst[q][:, k, il * P:(il + 1) * P],
                        wv_sb[:, k, :],
                        start=(k == 0 and sub == 0), stop=False)
                    yield
            nc.tensor.matmul(ps, ones_sb[:, :], bv_sb[:, :],
                             start=False, stop=True)
            yield
            for sub in range(2):
                i = st2 * 2 + sub
                nc.vector.memset(v_sb[i][:, :, DK], 1.0)
                nc.vector.tensor_copy(
                    v_sb[i][:, :, 0:DK],
                    ps[:, sub * CW:(sub + 1) * CW].rearrange(
                        "p (h d) -> p h d", h=HC))

        def wo_unit(m, ncol, tail=False):
            osb = osb_pool.tile([P, LCW], F32, tag="osb")
            wps = misc_ps.tile([P, LCW], F32, tag="m", name=f"wo{m}{ncol}")
            for p in range(2):
                nc.tensor.matmul(
                    wps, ot_sb[p][:, m * P:(m + 1) * P],
                    wo_sb[:, p, ncol * LCW:(ncol + 1) * LCW],
                    start=(p == 0), stop=(p == 1))
                yield
            if tail:
                # epilogue: exps are done, so the scalar engine and the Act
                # HWDGE ring are free; keeps the sync ring clear for the next
                # body's staging
                nc.scalar.copy(osb, wps)
                nc.scalar.dma_start(
                    out=aps["out_p"][m * P:(m + 1) * P,
                                     ncol * LCW:(ncol + 1) * LCW],
                    in_=osb)
            else:
                nc.vector.tensor_copy(osb, wps)
                nc.sync.dma_start(
                    out=aps["out_p"][m * P:(m + 1) * P,
                                     ncol * LCW:(ncol + 1) * LCW],
                    in_=osb)

        # ===== filler scheduler =====
        class Fillers:
            def __init__(self):
                self.q = []          # (label, gen) in deadline order
                self.done = set()

            def add(self, label, gen):
                self.q.append((label, gen))

            def pull(self, n):
                done = 0
                while done < n and self.q:
                    try:
                        next(self.q[0][1])
                        done += 1
                    except StopIteration:
                        self.done.add(self.q[0][0])
                        self.q.pop(0)

            def ensure(self, label):
                # PE executes in program order: every unit a consumer reads
                # from must be fully emitted before that consumer. Queue is
                # deadline-ordered, so flushing the prefix is safe.
                if label in self.done:
                    return
                while self.q:
                    lbl = self.q[0][0]
                    self.pull(1000000)
                    if lbl == label:
                        return

            def flush(self):
                while self.q:
                    self.pull(1000000)

        fillers = Fillers()
        wo_fill = Fillers()

        # ===== attention chunk =====
        def attn_chunk(j, proj_per_step, wo_per_step):
            n_i = 4 * j + 4
            fillers.ensure(("q", j))
            for p in range(2):
                ps_o = [ps_o_pool.tile([DK + 1, LCW], F32, tag="pso",
                                       name=f"pso{j}{p}{e}") for e in range(2)]
                ets = {}
                for ii in range(n_i + LAG):
                    if ii < n_i:
                        i = ii
                        d = max(0, i - 4 * j)
                        diag = i >= 4 * j
                        fillers.ensure(("k", i // 4))
                        et = et_pool.tile([P, 2 * LCW], BF16, tag="et")
                        for e in range(2):
                            ps_s = ps_s_pool.tile([P, LCW], F32, tag="ps",
                                                  name=f"pss{e}")
                            nc.tensor.matmul(
                                ps_s[:, d * P:],
                                kt_sb[p][e * DK:(e + 1) * DK,
                                         i * P:(i + 1) * P],
                                qt_sb[p][e * DK:(e + 1) * DK,
                                         j * LCW + d * P:(j + 1) * LCW],
                                start=True, stop=not diag)
                            if diag:
                                nc.tensor.matmul(
                                    ps_s[:, d * P:(d + 1) * P],
                                    ident_sb[:, :], mtri_sb[:, :],
                                    start=False, stop=True)
                            nc.scalar.activation(
                                et[:, e * LCW + d * P:(e + 1) * LCW],
                                ps_s[:, d * P:],
                                mybir.ActivationFunctionType.Exp,
                                scale=0.125,
                            )
                        ets[i] = et
                    if ii >= LAG:
                        i = ii - LAG
                        d = max(0, i - 4 * j)
                        fillers.ensure(("v", i // 2))
                        et = ets.pop(i)
                        for e in range(2):
                            h = 2 * p + e
                            nc.tensor.matmul(
                                ps_o[e][:, d * P:],
                                v_sb[i][:, h, :],
                                et[:, e * LCW + d * P:(e + 1) * LCW],
                                start=(i == 0), stop=(i == n_i - 1),
                            )
                    fillers.pull(proj_per_step)
                    wo_fill.pull(wo_per_step)
                # normalize: rows 0:64 = O^T unnormalized, row 64 = rowsum
                for e in range(2):
                    rec = recip_pool.tile([DK + 1, LCW], F32R, tag="rc")
                    nc.vector.reciprocal(rec[DK:DK + 1, :],
                                         ps_o[e][DK:DK + 1, :])
                    rb_sb = rbsb_pool.tile([DK, LCW], F32, tag="rbsb")
                    if NORM_BCAST:
                        # stride-0 SWDGE DMA broadcast (off the tensor engine)
                        rsrc = rec[DK:DK + 1, :]
                        nc.gpsimd.dma_start(
                            out=rb_sb,
                            in_=BassAP(rsrc.tensor, rsrc.offset,
                                       [list(rsrc.ap[0]), [0, DK], [1, LCW]]))
                    else:
                        rb_ps = misc_ps.tile([DK, LCW], F32, tag="m", name="rb")
                        nc.tensor.matmul(rb_ps, onesatt[DK:DK + 1, :],
                                         rec[DK:DK + 1, :],
                                         start=True, stop=True)
                        # DVE reads at most one PSUM operand: stage in SBUF
                        nc.vector.tensor_copy(rb_sb, rb_ps[:])
                    nc.vector.tensor_mul(
                        ot_sb[p][e * DK:(e + 1) * DK, j * LCW:(j + 1) * LCW],
                        ps_o[e][0:DK, :],
                        rb_sb[:],
                    )

        # ===== schedule =====
        # tiny loads off the critical path (SWDGE)
        nc.gpsimd.dma_start(out=bv_sb, in_=aps["bv2"])
        nc.gpsimd.dma_start(out=mtri_sb, in_=aps["mtri"])
        nc.gpsimd.dma_start(out=ident_sb, in_=aps["ident"])
        for p in range(2):
            nc.gpsimd.dma_start(out=bq_sb[p], in_=aps["bq3"][p])
            nc.gpsimd.dma_start(out=bk_sb[p], in_=aps["bk3"][p])
        # prologue data on the two HWDGE rings in consumption order, per-k
        # split; later quarters as single big SWDGE DMAs (issue cost hidden
        # under compute).
        nc.sync.dma_start(out=wv_sb, in_=aps["wv"])
        stage_q(xv_st, aps["xv_t"], 0, "xv", nc.sync, split=True)
        nc.sync.dma_start(out=wq_sb, in_=aps["wq"])
        stage_q(xq_st, aps["xq_t"], 0, "xq", nc.sync)
        nc.sync.dma_start(out=wo_sb, in_=aps["wo"])
        # second HWDGE ring (Activation) carries K's early tiles in parallel
        nc.scalar.dma_start(out=wk_sb, in_=aps["wk"])
        stage_q(xk_st, aps["xk_t"], 0, "xk", nc.scalar)
        stage_q(xq_st, aps["xq_t"], 1, "xq", nc.sync)
        stage_q(xk_st, aps["xk_t"], 1, "xk", nc.sync)
        stage_q(xv_st, aps["xv_t"], 1, "xv", nc.sync)
        for q in range(2, 4):
            stage_q(xq_st, aps["xq_t"], q, "xq", nc.sync)
            stage_q(xk_st, aps["xk_t"], q, "xk", nc.sync)
            stage_q(xv_st, aps["xv_t"], q, "xv", nc.sync)

        def run_all(gen):
            for _ in gen:
                pass

        # prologue: exactly what chunk 0 needs (s 0:512, l 0:512)
        run_all(v_unit(0, xv_st))
        run_all(v_unit(1, xv_st))
        for p in range(2):
            run_all(qk_unit(wq_sb, bq_sb, qt_sb, p, 0, xq_st))
        for p in range(2):
            run_all(qk_unit(wk_sb, bk_sb, kt_sb, p, 0, xk_st))

        # fillers in deadline order; labels let consumers flush JIT
        fillers.add(("q", 0), qk_unit(wq_sb, bq_sb, qt_sb, 0, 1, xq_st))
        fillers.add(("q", 1), qk_unit(wq_sb, bq_sb, qt_sb, 1, 1, xq_st))
        fillers.add(("k", 0), qk_unit(wk_sb, bk_sb, kt_sb, 0, 1, xk_st))
        fillers.add(("k", 1), qk_unit(wk_sb, bk_sb, kt_sb, 1, 1, xk_st))
        fillers.add(("v", 2), v_unit(2, xv_st))
        fillers.add(("v", 3), v_unit(3, xv_st))
        fillers.add(("q2a",), qk_unit(wq_sb, bq_sb, qt_sb, 0, 2, xq_st))
        fillers.add(("q", 2), qk_unit(wq_sb, bq_sb, qt_sb, 1, 2, xq_st))
        fillers.add(("k2a",), qk_unit(wk_sb, bk_sb, kt_sb, 0, 2, xk_st))
        fillers.add(("k", 2), qk_unit(wk_sb, bk_sb, kt_sb, 1, 2, xk_st))
        fillers.add(("v", 4), v_unit(4, xv_st))
        fillers.add(("v", 5), v_unit(5, xv_st))
        fillers.add(("q3a",), qk_unit(wq_sb, bq_sb, qt_sb, 0, 3, xq_st))
        fillers.add(("q", 3), qk_unit(wq_sb, bq_sb, qt_sb, 1, 3, xq_st))
        fillers.add(("k3a",), qk_unit(wk_sb, bk_sb, kt_sb, 0, 3, xk_st))
        fillers.add(("k", 3), qk_unit(wk_sb, bk_sb, kt_sb, 1, 3, xk_st))
        fillers.add(("v", 6), v_unit(6, xv_st))
        fillers.add(("v", 7), v_unit(7, xv_st))
        # chunk 0/1 reuse prologue tiles for their early labels
        fillers.done.update([("q", 0), ("k", 0), ("v", 0), ("v", 1)])

        def relabel():
            pass

        attn_chunk(0, 3, 0)
        for m in range(0, 4):
            for ncol in range(2):
                wo_fill.add(("wo", m, ncol), wo_unit(m, ncol))
        attn_chunk(1, 3, 1)
        for m in range(4, 8):
            for ncol in range(2):
                wo_fill.add(("wo", m, ncol), wo_unit(m, ncol))
        attn_chunk(2, 3, 1)
        for m in range(8, 12):
            for ncol in range(2):
                wo_fill.add(("wo", m, ncol), wo_unit(m, ncol))
        attn_chunk(3, 2, 2)
        wo_fill.flush()
        fillers.flush()
        for m in range(12, 16):
            for ncol in range(2):
                run_all(wo_unit(m, ncol))


# ---------------------------------------------------------------------------
_NC = None


def get_nc():
    global _NC
    if _NC is None:
        install_legalizer()
        _NC = build()
    return _NC


NPBF = mybir.dt.np(BF16)


def make_in_maps(queries, keys, values, Wq, bq, Wk, bk, Wv, bv, Wo, bo):
    kk, ll = np.meshgrid(np.arange(P), np.arange(P), indexing="ij")
    mtri = np.where(kk > ll, np.float32(NEG), np.float32(0.0)).astype(NPBF)
    ident = np.eye(P, dtype=np.float32).astype(NPBF)
    xts = {}
    for b in range(2):
        xts[b] = (
            np.ascontiguousarray(np.asarray(queries)[b].T).astype(NPBF),
            np.ascontiguousarray(np.asarray(keys)[b].T).astype(NPBF),
            np.ascontiguousarray(np.asarray(values)[b].T).astype(NPBF),
        )
    in_maps = []
    for c in range(8):
        b, q = divmod(c, 4)
        cs = slice(CW * q, CW * (q + 1))
        xq_t, xk_t, xv_t = xts[b]
        in_maps.append({
            "xq_t": xq_t,
            "xk_t": xk_t,
            "xv_t": xv_t,
            "wq": np.ascontiguousarray(
                np.asarray(Wq)[:, cs].reshape(KC, P, CW).transpose(1, 0, 2)
            ).astype(NPBF),
            "wk": np.ascontiguousarray(
                np.asarray(Wk)[:, cs].reshape(KC, P, CW).transpose(1, 0, 2)
            ).astype(NPBF),
            "wv": np.ascontiguousarray(
                np.asarray(Wv)[:, cs].reshape(KC, P, CW).transpose(1, 0, 2)
            ).astype(NPBF),
            "wo": np.ascontiguousarray(
                np.asarray(Wo)[cs, :].reshape(2, P, D).transpose(1, 0, 2)
            ).astype(NPBF),
            "bq3": np.asarray(bq)[cs].reshape(2, P, 1).astype(np.float32).copy(),
            "bk3": np.asarray(bk)[cs].reshape(2, P, 1).astype(np.float32).copy(),
            "bv2": np.tile(np.asarray(bv)[cs], 2).reshape(1, 2 * CW).astype(NPBF).copy(),
            "mtri": mtri,
            "ident": ident,
        })
    return in_maps


def gather(results, bo):
    bo = np.asarray(bo, np.float32)
    outs = [np.asarray(results[c]["out_p"], np.float32) for c in range(8)]
    b0 = outs[0] + outs[1] + outs[2] + outs[3] + bo
    b1 = outs[4] + outs[5] + outs[6] + outs[7] + bo
    return np.stack([b0, b1], axis=0).astype(np.float32)


def kernel(queries, keys, values, Wq, bq, Wk, bk, Wv, bv, Wo, bo):
    from concourse.bass_utils import run_bass_kernel_spmd
    nc = get_nc()
    in_maps = make_in_maps(queries, keys, values, Wq, bq, Wk, bk, Wv, bv, Wo, bo)
    res = run_bass_kernel_spmd(nc, in_maps, list(range(8)), trace=False)
    return gather(res.results, bo)



# revision 27
# speedup vs baseline: 1.1007x; 1.1007x over previous
"""Causal multi-head attention layer for Trainium2, SPMD across 8 NeuronCores.

Sharding: batch (B=2) x head-quads (16 heads -> 4 groups of 4) = 8 shards.
Core c handles batch c//4, heads 4*(c%4) .. 4*(c%4)+3.

v2: bf16 end-to-end (host-converted), causal mask folded into the scores
PSUM via an extra accumulating matmul (identity @ mask-triangle), Q/K
projection PSUM->SBUF copies on the vector engine (scalar engine does exp
only), and a software-pipelined schedule: a minimal prologue computes just
the V/Q/K tiles the first attention chunk needs, and every remaining
projection / output-projection matmul is woven one instruction at a time
between attention steps so the tensor engine stays dense while the scalar
engine works through the softmax exps.

Per core:
  - host pre-transposes activations to [d_model, tokens] bf16
  - Q^T,K^T projected as [dk*2, L] pairs (2 heads stacked on partitions),
    V projected in natural [S, dk] layout with a ones column appended
  - scores computed transposed [S, L]; mask as -1e30 matmul into PSUM;
    exp on the scalar engine; E @ [V|1] yields O^T plus softmax row-sums;
    normalization via reciprocal + ones-row matmul broadcast
  - output projection accumulates head pairs; host sums the 4 partial
    projections per batch and adds bo.
"""
import json

import numpy as np

import concourse.bass as bass
import concourse.mybir as mybir
import concourse.tile as tile
from concourse.ap import AP as BassAP

F32 = mybir.dt.float32
F32R = mybir.dt.float32r
BF16 = mybir.dt.bfloat16

D = 1024        # d_model
T = 2048        # tokens (L = S)
HC = 4          # heads per core
CW = 256        # projection cols per core (HC * 64)
KC = 8          # k chunks of 128 over D
NJ = 4          # l-chunks of 512
LCW = 512       # l chunk width
NST = 16        # s tiles of 128
P = 128
DK = 64
LAG = 3
NEG = -1e30
NORM_BCAST = False  # 1/r broadcast: False=PE matmul, True=SWDGE stride-0 DMA


# ---------------------------------------------------------------------------
# walrus in this container allows at most ONE sync-wait command per
# instruction; split extras onto preceding NoOps on the same engine
# (sequencers execute in order, so semantics are identical).
_orig_to_json_bytes = bass.Bass.to_json_bytes
_CTR = [0]


def _legalize(bir):
    for fn in bir.get("functions", []):
        for bb in fn.get("blocks", []):
            insts = bb.get("instructions", [])
            if not any(
                len((i.get("sync_info") or {}).get("on_wait") or []) > 1
                for i in insts
            ):
                continue
            out = []
            for inst in insts:
                si = inst.get("sync_info")
                waits = (si or {}).get("on_wait") or []
                if len(waits) > 1:
                    for w in waits[:-1]:
                        _CTR[0] += 1
                        nop = {
                            "engine": inst["engine"],
                            "ins": [],
                            "outs": [],
                            "name": f"lw-nop-{_CTR[0]}",
                            "opcode": "NoOp",
                            "sync_info": {"on_update": [], "on_wait": [w]},
                        }
                        if "debug" in inst:
                            nop["debug"] = inst["debug"]
                        out.append(nop)
                    si["on_wait"] = [waits[-1]]
                out.append(inst)
            bb["instructions"] = out
    return bir


def _patched_to_json_bytes(self):
    bir = json.loads(_orig_to_json_bytes(self))
    return json.dumps(_legalize(bir)).encode()


def install_legalizer():
    bass.Bass.to_json_bytes = _patched_to_json_bytes


# ---------------------------------------------------------------------------
def build(repeat=1):
    nc = bass.Bass("TRN2", target_bir_lowering=False, debug=False, num_devices=8)
    aps = {}
    for nm, shp, dt in [
        ("xq_t", [D, T], BF16), ("xk_t", [D, T], BF16), ("xv_t", [D, T], BF16),
        ("wq", [P, KC, CW], BF16), ("wk", [P, KC, CW], BF16),
        ("wv", [P, KC, CW], BF16),
        ("wo", [P, 2, D], BF16), ("bq3", [2, P, 1], F32), ("bk3", [2, P, 1], F32),
        ("bv2", [1, 2 * CW], BF16), ("mtri", [P, P], BF16),
        ("ident", [P, P], BF16),
    ]:
        aps[nm] = nc.dram_tensor(nm, shp, dt, kind="ExternalInput").ap()
    aps["out_p"] = nc.dram_tensor("out_p", [T, D], F32, kind="ExternalOutput").ap()

    with tile.TileContext(nc) as tc:
        for _ in range(repeat):
            _body(tc, nc, aps)
    return nc


def _body(tc, nc, aps):
    from contextlib import ExitStack
    ctx = ExitStack()
    with ctx:
        ctx.enter_context(nc.allow_low_precision(
            reason="bf16 compute is intentional; fp32 accumulate in PSUM"))
        singles = ctx.enter_context(tc.tile_pool(name="singles", bufs=1))
        xt_pool = ctx.enter_context(tc.tile_pool(name="xt", bufs=12))
        et_pool = ctx.enter_context(tc.tile_pool(name="et", bufs=6))
        recip_pool = ctx.enter_context(tc.tile_pool(name="recip", bufs=2))
        rbsb_pool = ctx.enter_context(tc.tile_pool(name="rbsb", bufs=2))
        osb_pool = ctx.enter_context(tc.tile_pool(name="osb", bufs=4))

        wq_sb = singles.tile([P, KC, CW], BF16, tag="wq")
        wk_sb = singles.tile([P, KC, CW], BF16, tag="wk")
        wv_sb = singles.tile([P, KC, CW], BF16, tag="wv")
        wo_sb = singles.tile([P, 2, D], BF16, tag="wo")
        bq_sb = [singles.tile([P, 1], F32, tag=f"bq{p}", name=f"bq_sb{p}") for p in range(2)]
        bk_sb = [singles.tile([P, 1], F32, tag=f"bk{p}", name=f"bk_sb{p}") for p in range(2)]
        bv_sb = singles.tile([1, 2 * CW], BF16, tag="bv")
        mtri_sb = singles.tile([P, P], BF16, tag="mtri")
        ident_sb = singles.tile([P, P], BF16, tag="ident")
        ones_sb = singles.tile([1, P], BF16, tag="ones")
        nc.vector.memset(ones_sb, 1.0)
        onesatt = singles.tile([DK + 1, DK], F32R, tag="onesatt")
        ones_f32 = singles.tile([DK + 1, DK], F32, tag="ones_f32")
        nc.vector.memset(ones_f32[DK:DK + 1, :], 1.0)
        nc.vector.tensor_copy(onesatt[DK:DK + 1, :], ones_f32[DK:DK + 1, :])

        qt_sb = [singles.tile([P, T], BF16, tag=f"qt{p}", name=f"qt_sb{p}") for p in range(2)]
        kt_sb = [singles.tile([P, T], BF16, tag=f"kt{p}", name=f"kt_sb{p}") for p in range(2)]
        ot_sb = [singles.tile([P, T], BF16, tag=f"ot{p}", name=f"ot_sb{p}") for p in range(2)]
        v_sb = [singles.tile([P, HC, DK + 1], BF16, tag=f"v{i}", name=f"v_sb{i}")
                for i in range(NST)]

        # ===== staging: one [P, KC, 512] tile + one DMA per (input, quarter)
        xq_st = [None] * 4
        xk_st = [None] * 4
        xv_st = [None] * 4

        def stage_q(store, x_ap, q, tagpfx, engine, split=False):
            xt = xt_pool.tile([P, KC, LCW], BF16, tag="xt",
                              name=f"{tagpfx}{q}")
            src = x_ap.rearrange("(ko ki) t -> ki ko t", ki=P)[
                :, :, q * LCW:(q + 1) * LCW]
            if split:
                # per-k DMAs into one tile: fine-grained completion lets the
                # prologue matmuls start as soon as their k-chunk lands
                for k in range(KC):
                    engine.dma_start(out=xt[:, k, :], in_=src[:, k, :])
            else:
                engine.dma_start(out=xt, in_=src)
            store[q] = xt

        # ===== work units =====
        def qk_unit(w_sb, b_sb, dst, p, lc, xst, mps):
            ps = mps.tile([P, LCW], F32, tag="m", name=f"qk{p}{lc}")
            for k in range(KC):
                nc.tensor.matmul(
                    ps, w_sb[:, k, p * P:(p + 1) * P],
                    xst[lc][:, k, :],
                    start=(k == 0), stop=(k == KC - 1))
            nc.vector.tensor_scalar_add(
                dst[p][:, lc * LCW:(lc + 1) * LCW], ps, b_sb[p][:, 0:1])

        def v_unit(st2, xst, mps):
            ps = mps.tile([P, 2 * CW], F32, tag="m", name=f"v{st2}")
            q, qsub = divmod(st2, 2)
            for k in range(KC):
                for sub in range(2):
                    il = qsub * 2 + sub
                    nc.tensor.matmul(
                        ps[:, sub * CW:(sub + 1) * CW],
                        xst[q][:, k, il * P:(il + 1) * P],
                        wv_sb[:, k, :],
                        start=(k == 0 and sub == 0), stop=False)
            nc.tensor.matmul(ps, ones_sb[:, :], bv_sb[:, :],
                             start=False, stop=True)
            for sub in range(2):
                i = st2 * 2 + sub
                nc.vector.memset(v_sb[i][:, :, DK], 1.0)
                nc.vector.tensor_copy(
                    v_sb[i][:, :, 0:DK],
                    ps[:, sub * CW:(sub + 1) * CW].rearrange(
                        "p (h d) -> p h d", h=HC))

        def wo_unit(m, ncol, mps):
            osb = osb_pool.tile([P, LCW], F32, tag="osb")
            wps = mps.tile([P, LCW], F32, tag="m", name=f"wo{m}{ncol}")
            for p in range(2):
                nc.tensor.matmul(
                    wps, ot_sb[p][:, m * P:(m + 1) * P],
                    wo_sb[:, p, ncol * LCW:(ncol + 1) * LCW],
                    start=(p == 0), stop=(p == 1))
            # alternate evacuation engine + DMA ring for P3 parallelism
            if (2 * m + ncol) % 2 == 0:
                nc.vector.tensor_copy(osb, wps)
                nc.sync.dma_start(
                    out=aps["out_p"][m * P:(m + 1) * P,
                                     ncol * LCW:(ncol + 1) * LCW],
                    in_=osb)
            else:
                nc.scalar.copy(osb, wps)
                nc.scalar.dma_start(
                    out=aps["out_p"][m * P:(m + 1) * P,
                                     ncol * LCW:(ncol + 1) * LCW],
                    in_=osb)

        # ===== attention chunk (phase 2: deep ps_s rotation, no fillers) ===
        def attn_chunk(j, ps_s_pool, ps_o_pool):
            n_i = 4 * j + 4
            for p in range(2):
                ps_o = [ps_o_pool.tile([DK + 1, LCW], F32, tag="pso",
                                       name=f"pso{j}{p}{e}") for e in range(2)]
                ets = {}
                for ii in range(n_i + LAG):
                    if ii < n_i:
                        i = ii
                        d = max(0, i - 4 * j)
                        diag = i >= 4 * j
                        ps_s = ps_s_pool.tile([P, 2 * LCW], F32, tag="ps",
                                              name="pss")
                        for e in range(2):
                            nc.tensor.matmul(
                                ps_s[:, e * LCW + d * P:(e + 1) * LCW],
                                kt_sb[p][e * DK:(e + 1) * DK,
                                         i * P:(i + 1) * P],
                                qt_sb[p][e * DK:(e + 1) * DK,
                                         j * LCW + d * P:(j + 1) * LCW],
                                start=True, stop=not diag)
                            if diag:
                                nc.tensor.matmul(
                                    ps_s[:, e * LCW + d * P:
                                         e * LCW + (d + 1) * P],
                                    ident_sb[:, :], mtri_sb[:, :],
                                    start=False, stop=True)
                        et = et_pool.tile([P, 2 * LCW], BF16, tag="et")
                        nc.scalar.activation(
                            et.rearrange("p (e l) -> p e l", e=2)[:, :, d * P:],
                            ps_s.rearrange("p (e l) -> p e l", e=2)[:, :, d * P:],
                            mybir.ActivationFunctionType.Exp,
                            scale=0.125,
                        )
                        ets[i] = et
                    if ii >= LAG:
                        i = ii - LAG
                        d = max(0, i - 4 * j)
                        et = ets.pop(i)
                        for e in range(2):
                            h = 2 * p + e
                            nc.tensor.matmul(
                                ps_o[e][:, d * P:],
                                v_sb[i][:, h, :],
                                et[:, e * LCW + d * P:(e + 1) * LCW],
                                start=(i == 0), stop=(i == n_i - 1),
                            )
                # normalize: rows 0:64 = O^T unnormalized, row 64 = rowsum
                for e in range(2):
                    rec = recip_pool.tile([DK + 1, LCW], F32R, tag="rc")
                    nc.vector.reciprocal(rec[DK:DK + 1, :],
                                         ps_o[e][DK:DK + 1, :])
                    rb_sb = rbsb_pool.tile([DK, LCW], F32, tag="rbsb")
                    # rb matmul borrows a ps_s slot (scores stream has wound
                    # down by normalize time, so a slot is free)
                    rb_ps = ps_s_pool.tile([P, 2 * LCW], F32, tag="ps",
                                           name="pss")
                    nc.tensor.matmul(rb_ps[0:DK, 0:LCW],
                                     onesatt[DK:DK + 1, :],
                                     rec[DK:DK + 1, :],
                                     start=True, stop=True)
                    # DVE reads at most one PSUM operand: stage in SBUF
                    nc.vector.tensor_copy(rb_sb, rb_ps[0:DK, 0:LCW])
                    nc.vector.tensor_mul(
                        ot_sb[p][e * DK:(e + 1) * DK, j * LCW:(j + 1) * LCW],
                        ps_o[e][0:DK, :],
                        rb_sb[:],
                    )

        # ===== schedule =====
        # tiny loads off the critical path (SWDGE)
        nc.gpsimd.dma_start(out=bv_sb, in_=aps["bv2"])
        nc.gpsimd.dma_start(out=mtri_sb, in_=aps["mtri"])
        nc.gpsimd.dma_start(out=ident_sb, in_=aps["ident"])
        for p in range(2):
            nc.gpsimd.dma_start(out=bq_sb[p], in_=aps["bq3"][p])
            nc.gpsimd.dma_start(out=bk_sb[p], in_=aps["bk3"][p])
        # staging on the two HWDGE rings in consumption order, per-k split
        # for the first quarters so matmuls start as chunks land
        nc.sync.dma_start(out=wv_sb, in_=aps["wv"])
        stage_q(xv_st, aps["xv_t"], 0, "xv", nc.sync, split=True)
        nc.sync.dma_start(out=wq_sb, in_=aps["wq"])
        stage_q(xq_st, aps["xq_t"], 0, "xq", nc.sync)
        nc.sync.dma_start(out=wo_sb, in_=aps["wo"])
        # second HWDGE ring (Activation) carries K's early tiles in parallel
        nc.scalar.dma_start(out=wk_sb, in_=aps["wk"])
        stage_q(xk_st, aps["xk_t"], 0, "xk", nc.scalar)
        stage_q(xq_st, aps["xq_t"], 1, "xq", nc.sync)
        stage_q(xk_st, aps["xk_t"], 1, "xk", nc.sync)
        stage_q(xv_st, aps["xv_t"], 1, "xv", nc.sync)
        for q in range(2, 4):
            stage_q(xq_st, aps["xq_t"], q, "xq", nc.sync)
            stage_q(xk_st, aps["xk_t"], q, "xk", nc.sync)
            stage_q(xv_st, aps["xv_t"], q, "xv", nc.sync)

        # ===== phase 1: all projections, deep misc rotation ===============
        with tc.tile_pool(name="mps1", bufs=6, space="PSUM") as misc1:
            v_unit(0, xv_st, misc1)
            v_unit(1, xv_st, misc1)
            for p in range(2):
                qk_unit(wq_sb, bq_sb, qt_sb, p, 0, xq_st, misc1)
            for p in range(2):
                qk_unit(wk_sb, bk_sb, kt_sb, p, 0, xk_st, misc1)
            for lc in range(1, 4):
                for p in range(2):
                    qk_unit(wq_sb, bq_sb, qt_sb, p, lc, xq_st, misc1)
                for p in range(2):
                    qk_unit(wk_sb, bk_sb, kt_sb, p, lc, xk_st, misc1)
                v_unit(2 * lc, xv_st, misc1)
                v_unit(2 * lc + 1, xv_st, misc1)

        # ===== phase 2: attention, ps_s depth 3 ============================
        with tc.tile_pool(name="pss", bufs=3, space="PSUM") as ps_s_pool, \
                tc.tile_pool(name="pso", bufs=2, space="PSUM") as ps_o_pool:
            for j in range(4):
                attn_chunk(j, ps_s_pool, ps_o_pool)

        # ===== phase 3: output projection ==================================
        with tc.tile_pool(name="mps2", bufs=4, space="PSUM") as misc2:
            for m in range(16):
                for ncol in range(2):
                    wo_unit(m, ncol, misc2)


# ---------------------------------------------------------------------------
_NC = None


def get_nc():
    global _NC
    if _NC is None:
        install_legalizer()
        _NC = build()
    return _NC


NPBF = mybir.dt.np(BF16)


def make_in_maps(queries, keys, values, Wq, bq, Wk, bk, Wv, bv, Wo, bo):
    kk, ll = np.meshgrid(np.arange(P), np.arange(P), indexing="ij")
    mtri = np.where(kk > ll, np.float32(NEG), np.float32(0.0)).astype(NPBF)
    ident = np.eye(P, dtype=np.float32).astype(NPBF)
    xts = {}
    for b in range(2):
        xts[b] = (
            np.ascontiguousarray(np.asarray(queries)[b].T).astype(NPBF),
            np.ascontiguousarray(np.asarray(keys)[b].T).astype(NPBF),
            np.ascontiguousarray(np.asarray(values)[b].T).astype(NPBF),
        )
    in_maps = []
    for c in range(8):
        b, q = divmod(c, 4)
        cs = slice(CW * q, CW * (q + 1))
        xq_t, xk_t, xv_t = xts[b]
        in_maps.append({
            "xq_t": xq_t,
            "xk_t": xk_t,
            "xv_t": xv_t,
            "wq": np.ascontiguousarray(
                np.asarray(Wq)[:, cs].reshape(KC, P, CW).transpose(1, 0, 2)
            ).astype(NPBF),
            "wk": np.ascontiguousarray(
                np.asarray(Wk)[:, cs].reshape(KC, P, CW).transpose(1, 0, 2)
            ).astype(NPBF),
            "wv": np.ascontiguousarray(
                np.asarray(Wv)[:, cs].reshape(KC, P, CW).transpose(1, 0, 2)
            ).astype(NPBF),
            "wo": np.ascontiguousarray(
                np.asarray(Wo)[cs, :].reshape(2, P, D).transpose(1, 0, 2)
            ).astype(NPBF),
            "bq3": np.asarray(bq)[cs].reshape(2, P, 1).astype(np.float32).copy(),
            "bk3": np.asarray(bk)[cs].reshape(2, P, 1).astype(np.float32).copy(),
            "bv2": np.tile(np.asarray(bv)[cs], 2).reshape(1, 2 * CW).astype(NPBF).copy(),
            "mtri": mtri,
            "ident": ident,
        })
    return in_maps


def gather(results, bo):
    bo = np.asarray(bo, np.float32)
    outs = [np.asarray(results[c]["out_p"], np.float32) for c in range(8)]
    b0 = outs[0] + outs[1] + outs[2] + outs[3] + bo
    b1 = outs[4] + outs[5] + outs[6] + outs[7] + bo
    return np.stack([b0, b1], axis=0).astype(np.float32)


def kernel(queries, keys, values, Wq, bq, Wk, bk, Wv, bv, Wo, bo):
    from concourse.bass_utils import run_bass_kernel_spmd
    nc = get_nc()
    in_maps = make_in_maps(queries, keys, values, Wq, bq, Wk, bk, Wv, bv, Wo, bo)
    res = run_bass_kernel_spmd(nc, in_maps, list(range(8)), trace=False)
    return gather(res.results, bo)



# revision 30
# speedup vs baseline: 1.1215x; 1.0189x over previous
"""Causal multi-head attention layer for Trainium2, SPMD across 8 NeuronCores.

Sharding: batch (B=2) x head-quads (16 heads -> 4 groups of 4) = 8 shards.
Core c handles batch c//4, heads 4*(c%4) .. 4*(c%4)+3.

v2: bf16 end-to-end (host-converted), causal mask folded into the scores
PSUM via an extra accumulating matmul (identity @ mask-triangle), Q/K
projection PSUM->SBUF copies on the vector engine (scalar engine does exp
only), and a software-pipelined schedule: a minimal prologue computes just
the V/Q/K tiles the first attention chunk needs, and every remaining
projection / output-projection matmul is woven one instruction at a time
between attention steps so the tensor engine stays dense while the scalar
engine works through the softmax exps.

Per core:
  - host pre-transposes activations to [d_model, tokens] bf16
  - Q^T,K^T projected as [dk*2, L] pairs (2 heads stacked on partitions),
    V projected in natural [S, dk] layout with a ones column appended
  - scores computed transposed [S, L]; mask as -1e30 matmul into PSUM;
    exp on the scalar engine; E @ [V|1] yields O^T plus softmax row-sums;
    normalization via reciprocal + ones-row matmul broadcast
  - output projection accumulates head pairs; host sums the 4 partial
    projections per batch and adds bo.
"""
import json

import numpy as np

import concourse.bass as bass
import concourse.mybir as mybir
import concourse.tile as tile
from concourse.ap import AP as BassAP

F32 = mybir.dt.float32
F32R = mybir.dt.float32r
BF16 = mybir.dt.bfloat16

D = 1024        # d_model
T = 2048        # tokens (L = S)
HC = 4          # heads per core
CW = 256        # projection cols per core (HC * 64)
KC = 8          # k chunks of 128 over D
NJ = 4          # l-chunks of 512
LCW = 512       # l chunk width
NST = 16        # s tiles of 128
P = 128
DK = 64
LAG = 3
NEG = -1e30
NORM_BCAST = False  # 1/r broadcast: False=PE matmul, True=SWDGE stride-0 DMA


# ---------------------------------------------------------------------------
# walrus in this container allows at most ONE sync-wait command per
# instruction; split extras onto preceding NoOps on the same engine
# (sequencers execute in order, so semantics are identical).
_orig_to_json_bytes = bass.Bass.to_json_bytes
_CTR = [0]


def _legalize(bir):
    for fn in bir.get("functions", []):
        for bb in fn.get("blocks", []):
            insts = bb.get("instructions", [])
            if not any(
                len((i.get("sync_info") or {}).get("on_wait") or []) > 1
                for i in insts
            ):
                continue
            out = []
            for inst in insts:
                si = inst.get("sync_info")
                waits = (si or {}).get("on_wait") or []
                if len(waits) > 1:
                    for w in waits[:-1]:
                        _CTR[0] += 1
                        nop = {
                            "engine": inst["engine"],
                            "ins": [],
                            "outs": [],
                            "name": f"lw-nop-{_CTR[0]}",
                            "opcode": "NoOp",
                            "sync_info": {"on_update": [], "on_wait": [w]},
                        }
                        if "debug" in inst:
                            nop["debug"] = inst["debug"]
                        out.append(nop)
                    si["on_wait"] = [waits[-1]]
                out.append(inst)
            bb["instructions"] = out
    return bir


def _patched_to_json_bytes(self):
    bir = json.loads(_orig_to_json_bytes(self))
    return json.dumps(_legalize(bir)).encode()


def install_legalizer():
    bass.Bass.to_json_bytes = _patched_to_json_bytes


# ---------------------------------------------------------------------------
def build(repeat=1):
    nc = bass.Bass("TRN2", target_bir_lowering=False, debug=False, num_devices=8)
    aps = {}
    for nm, shp, dt in [
        ("xq_t", [D, T], BF16), ("xk_t", [D, T], BF16), ("xv_t", [D, T], BF16),
        ("wq", [P, KC, CW], BF16), ("wk", [P, KC, CW], BF16),
        ("wv", [P, KC, CW], BF16),
        ("wo", [P, 2, D], BF16), ("bq3", [2, P, 1], F32), ("bk3", [2, P, 1], F32),
        ("bv2", [1, 2 * CW], BF16), ("mtri", [P, P], BF16),
        ("ident", [P, P], BF16),
    ]:
        aps[nm] = nc.dram_tensor(nm, shp, dt, kind="ExternalInput").ap()
    aps["out_p"] = nc.dram_tensor("out_p", [T, D], F32, kind="ExternalOutput").ap()

    with tile.TileContext(nc) as tc:
        for _ in range(repeat):
            _body(tc, nc, aps)
    return nc


def _body(tc, nc, aps):
    from contextlib import ExitStack
    ctx = ExitStack()
    with ctx:
        ctx.enter_context(nc.allow_low_precision(
            reason="bf16 compute is intentional; fp32 accumulate in PSUM"))
        singles = ctx.enter_context(tc.tile_pool(name="singles", bufs=1))
        xt_pool = ctx.enter_context(tc.tile_pool(name="xt", bufs=12))
        et_pool = ctx.enter_context(tc.tile_pool(name="et", bufs=6))
        recip_pool = ctx.enter_context(tc.tile_pool(name="recip", bufs=2))
        rbsb_pool = ctx.enter_context(tc.tile_pool(name="rbsb", bufs=2))
        osb_pool = ctx.enter_context(tc.tile_pool(name="osb", bufs=4))

        wq_sb = singles.tile([P, KC, CW], BF16, tag="wq")
        wk_sb = singles.tile([P, KC, CW], BF16, tag="wk")
        wv_sb = singles.tile([P, KC, CW], BF16, tag="wv")
        wo_sb = singles.tile([P, 2, D], BF16, tag="wo")
        bq_sb = [singles.tile([P, 1], F32, tag=f"bq{p}", name=f"bq_sb{p}") for p in range(2)]
        bk_sb = [singles.tile([P, 1], F32, tag=f"bk{p}", name=f"bk_sb{p}") for p in range(2)]
        bv_sb = singles.tile([1, 2 * CW], BF16, tag="bv")
        mtri_sb = singles.tile([P, P], BF16, tag="mtri")
        ident_sb = singles.tile([P, P], BF16, tag="ident")
        ones_sb = singles.tile([1, P], BF16, tag="ones")
        nc.vector.memset(ones_sb, 1.0)
        onesatt = singles.tile([DK + 1, DK], F32R, tag="onesatt")
        ones_f32 = singles.tile([DK + 1, DK], F32, tag="ones_f32")
        nc.vector.memset(ones_f32[DK:DK + 1, :], 1.0)
        nc.vector.tensor_copy(onesatt[DK:DK + 1, :], ones_f32[DK:DK + 1, :])

        qt_sb = [singles.tile([P, T], BF16, tag=f"qt{p}", name=f"qt_sb{p}") for p in range(2)]
        kt_sb = [singles.tile([P, T], BF16, tag=f"kt{p}", name=f"kt_sb{p}") for p in range(2)]
        ot_sb = [singles.tile([P, T], BF16, tag=f"ot{p}", name=f"ot_sb{p}") for p in range(2)]
        v_sb = [singles.tile([P, HC, DK + 1], BF16, tag=f"v{i}", name=f"v_sb{i}")
                for i in range(NST)]

        # ===== staging: one [P, KC, 512] tile + one DMA per (input, quarter)
        xq_st = [None] * 4
        xk_st = [None] * 4
        xv_st = [None] * 4

        def stage_q(store, x_ap, q, tagpfx, engine, split=False):
            xt = xt_pool.tile([P, KC, LCW], BF16, tag="xt",
                              name=f"{tagpfx}{q}")
            src = x_ap.rearrange("(ko ki) t -> ki ko t", ki=P)[
                :, :, q * LCW:(q + 1) * LCW]
            if split:
                # per-k DMAs into one tile: fine-grained completion lets the
                # prologue matmuls start as soon as their k-chunk lands
                for k in range(KC):
                    engine.dma_start(out=xt[:, k, :], in_=src[:, k, :])
            else:
                engine.dma_start(out=xt, in_=src)
            store[q] = xt

        # ===== work units =====
        def qk_unit(w_sb, b_sb, dst, p, lc, xst, mps):
            ps = mps.tile([P, LCW], F32, tag="m", name=f"qk{p}{lc}")
            for k in range(KC):
                nc.tensor.matmul(
                    ps, w_sb[:, k, p * P:(p + 1) * P],
                    xst[lc][:, k, :],
                    start=(k == 0), stop=(k == KC - 1))
            nc.vector.tensor_scalar_add(
                dst[p][:, lc * LCW:(lc + 1) * LCW], ps, b_sb[p][:, 0:1])

        def v_unit(st2, xst, mps):
            ps = mps.tile([P, 2 * CW], F32, tag="m", name=f"v{st2}")
            q, qsub = divmod(st2, 2)
            for k in range(KC):
                for sub in range(2):
                    il = qsub * 2 + sub
                    nc.tensor.matmul(
                        ps[:, sub * CW:(sub + 1) * CW],
                        xst[q][:, k, il * P:(il + 1) * P],
                        wv_sb[:, k, :],
                        start=(k == 0 and sub == 0), stop=False)
            nc.tensor.matmul(ps, ones_sb[:, :], bv_sb[:, :],
                             start=False, stop=True)
            for sub in range(2):
                i = st2 * 2 + sub
                nc.vector.memset(v_sb[i][:, :, DK], 1.0)
                nc.vector.tensor_copy(
                    v_sb[i][:, :, 0:DK],
                    ps[:, sub * CW:(sub + 1) * CW].rearrange(
                        "p (h d) -> p h d", h=HC))

        def wo_unit(m, ncol, mps):
            osb = osb_pool.tile([P, LCW], F32, tag="osb")
            wps = mps.tile([P, LCW], F32, tag="m", name=f"wo{m}{ncol}")
            for p in range(2):
                nc.tensor.matmul(
                    wps, ot_sb[p][:, m * P:(m + 1) * P],
                    wo_sb[:, p, ncol * LCW:(ncol + 1) * LCW],
                    start=(p == 0), stop=(p == 1))
            # alternate evacuation engine + DMA ring for P3 parallelism
            if (2 * m + ncol) % 2 == 0:
                nc.vector.tensor_copy(osb, wps)
                nc.sync.dma_start(
                    out=aps["out_p"][m * P:(m + 1) * P,
                                     ncol * LCW:(ncol + 1) * LCW],
                    in_=osb)
            else:
                nc.scalar.copy(osb, wps)
                nc.scalar.dma_start(
                    out=aps["out_p"][m * P:(m + 1) * P,
                                     ncol * LCW:(ncol + 1) * LCW],
                    in_=osb)

        # ===== attention chunk (phase 2: deep ps_s rotation, no fillers) ===
        # Normalize tails are deferred: the reciprocal is issued at the chunk
        # boundary (DVE-only), but the PE rb-matmul and the DVE copy/mul are
        # spliced into the NEXT chunk's steps so their waits are pre-satisfied
        # when the in-order PE reaches them.
        pending_norm = []

        def flush_norms():
            for fn in pending_norm:
                fn()
            pending_norm.clear()

        def attn_chunk(j, ps_s_pool, ps_o_pool):
            n_i = 4 * j + 4
            for p in range(2):
                ps_o = [ps_o_pool.tile([DK + 1, LCW], F32, tag="pso",
                                       name=f"pso{j}{p}{e}") for e in range(2)]
                ets = {}
                for ii in range(n_i + LAG):
                    if ii == 2:
                        flush_norms()
                    if ii < n_i:
                        i = ii
                        d = max(0, i - 4 * j)
                        diag = i >= 4 * j
                        ps_s = ps_s_pool.tile([P, 2 * LCW], F32, tag="ps",
                                              name="pss")
                        for e in range(2):
                            nc.tensor.matmul(
                                ps_s[:, e * LCW + d * P:(e + 1) * LCW],
                                kt_sb[p][e * DK:(e + 1) * DK,
                                         i * P:(i + 1) * P],
                                qt_sb[p][e * DK:(e + 1) * DK,
                                         j * LCW + d * P:(j + 1) * LCW],
                                start=True, stop=not diag)
                            if diag:
                                nc.tensor.matmul(
                                    ps_s[:, e * LCW + d * P:
                                         e * LCW + (d + 1) * P],
                                    ident_sb[:, :], mtri_sb[:, :],
                                    start=False, stop=True)
                        et = et_pool.tile([P, 2 * LCW], BF16, tag="et")
                        nc.scalar.activation(
                            et.rearrange("p (e l) -> p e l", e=2)[:, :, d * P:],
                            ps_s.rearrange("p (e l) -> p e l", e=2)[:, :, d * P:],
                            mybir.ActivationFunctionType.Exp,
                            scale=0.125,
                        )
                        ets[i] = et
                    if ii >= LAG:
                        i = ii - LAG
                        d = max(0, i - 4 * j)
                        et = ets.pop(i)
                        for e in range(2):
                            h = 2 * p + e
                            nc.tensor.matmul(
                                ps_o[e][:, d * P:],
                                v_sb[i][:, h, :],
                                et[:, e * LCW + d * P:(e + 1) * LCW],
                                start=(i == 0), stop=(i == n_i - 1),
                            )
                # normalize: rows 0:64 = O^T unnormalized, row 64 = rowsum.
                # recip now; rb matmul + copy + mul deferred (see flush_norms)
                recs = []
                for e in range(2):
                    rec = recip_pool.tile([DK + 1, LCW], F32R, tag="rc")
                    nc.vector.reciprocal(rec[DK:DK + 1, :],
                                         ps_o[e][DK:DK + 1, :])
                    recs.append(rec)

                def norm_tail(j=j, p=p, ps_o=ps_o, recs=recs):
                    for e in range(2):
                        rb_sb = rbsb_pool.tile([DK, LCW], F32, tag="rbsb")
                        # rb matmul borrows a ps_s slot (free by now)
                        rb_ps = ps_s_pool.tile([P, 2 * LCW], F32, tag="ps",
                                               name="pss")
                        nc.tensor.matmul(rb_ps[0:DK, 0:LCW],
                                         onesatt[DK:DK + 1, :],
                                         recs[e][DK:DK + 1, :],
                                         start=True, stop=True)
                        # DVE reads at most one PSUM operand: stage in SBUF
                        nc.vector.tensor_copy(rb_sb, rb_ps[0:DK, 0:LCW])
                        nc.vector.tensor_mul(
                            ot_sb[p][e * DK:(e + 1) * DK,
                                     j * LCW:(j + 1) * LCW],
                            ps_o[e][0:DK, :],
                            rb_sb[:],
                        )

                pending_norm.append(norm_tail)

        # ===== schedule =====
        # tiny loads off the critical path (SWDGE)
        nc.gpsimd.dma_start(out=bv_sb, in_=aps["bv2"])
        nc.gpsimd.dma_start(out=mtri_sb, in_=aps["mtri"])
        nc.gpsimd.dma_start(out=ident_sb, in_=aps["ident"])
        for p in range(2):
            nc.gpsimd.dma_start(out=bq_sb[p], in_=aps["bq3"][p])
            nc.gpsimd.dma_start(out=bk_sb[p], in_=aps["bk3"][p])
        # staging on the two HWDGE rings in consumption order, per-k split
        # for the first quarters so matmuls start as chunks land
        nc.sync.dma_start(out=wv_sb, in_=aps["wv"])
        stage_q(xv_st, aps["xv_t"], 0, "xv", nc.sync, split=True)
        nc.sync.dma_start(out=wq_sb, in_=aps["wq"])
        stage_q(xq_st, aps["xq_t"], 0, "xq", nc.sync)
        nc.sync.dma_start(out=wo_sb, in_=aps["wo"])
        # second HWDGE ring (Activation) carries K's early tiles in parallel
        nc.scalar.dma_start(out=wk_sb, in_=aps["wk"])
        stage_q(xk_st, aps["xk_t"], 0, "xk", nc.scalar)
        stage_q(xq_st, aps["xq_t"], 1, "xq", nc.sync)
        stage_q(xk_st, aps["xk_t"], 1, "xk", nc.sync)
        stage_q(xv_st, aps["xv_t"], 1, "xv", nc.sync)
        for q in range(2, 4):
            stage_q(xq_st, aps["xq_t"], q, "xq", nc.sync)
            stage_q(xk_st, aps["xk_t"], q, "xk", nc.sync)
            stage_q(xv_st, aps["xv_t"], q, "xv", nc.sync)

        # ===== phase 1: all projections, deep misc rotation ===============
        with tc.tile_pool(name="mps1", bufs=6, space="PSUM") as misc1:
            v_unit(0, xv_st, misc1)
            v_unit(1, xv_st, misc1)
            for p in range(2):
                qk_unit(wq_sb, bq_sb, qt_sb, p, 0, xq_st, misc1)
            for p in range(2):
                qk_unit(wk_sb, bk_sb, kt_sb, p, 0, xk_st, misc1)
            for lc in range(1, 4):
                for p in range(2):
                    qk_unit(wq_sb, bq_sb, qt_sb, p, lc, xq_st, misc1)
                for p in range(2):
                    qk_unit(wk_sb, bk_sb, kt_sb, p, lc, xk_st, misc1)
                v_unit(2 * lc, xv_st, misc1)
                v_unit(2 * lc + 1, xv_st, misc1)

        # ===== phase 2: attention, ps_s depth 3 ============================
        with tc.tile_pool(name="pss", bufs=3, space="PSUM") as ps_s_pool, \
                tc.tile_pool(name="pso", bufs=2, space="PSUM") as ps_o_pool:
            for j in range(4):
                attn_chunk(j, ps_s_pool, ps_o_pool)
            flush_norms()

        # ===== phase 3: output projection ==================================
        with tc.tile_pool(name="mps2", bufs=4, space="PSUM") as misc2:
            for m in range(16):
                for ncol in range(2):
                    wo_unit(m, ncol, misc2)


# ---------------------------------------------------------------------------
_NC = None


def get_nc():
    global _NC
    if _NC is None:
        install_legalizer()
        _NC = build()
    return _NC


NPBF = mybir.dt.np(BF16)


def make_in_maps(queries, keys, values, Wq, bq, Wk, bk, Wv, bv, Wo, bo):
    kk, ll = np.meshgrid(np.arange(P), np.arange(P), indexing="ij")
    mtri = np.where(kk > ll, np.float32(NEG), np.float32(0.0)).astype(NPBF)
    ident = np.eye(P, dtype=np.float32).astype(NPBF)
    xts = {}
    for b in range(2):
        xts[b] = (
            np.ascontiguousarray(np.asarray(queries)[b].T).astype(NPBF),
            np.ascontiguousarray(np.asarray(keys)[b].T).astype(NPBF),
            np.ascontiguousarray(np.asarray(values)[b].T).astype(NPBF),
        )
    in_maps = []
    for c in range(8):
        b, q = divmod(c, 4)
        cs = slice(CW * q, CW * (q + 1))
        xq_t, xk_t, xv_t = xts[b]
        in_maps.append({
            "xq_t": xq_t,
            "xk_t": xk_t,
            "xv_t": xv_t,
            "wq": np.ascontiguousarray(
                np.asarray(Wq)[:, cs].reshape(KC, P, CW).transpose(1, 0, 2)
            ).astype(NPBF),
            "wk": np.ascontiguousarray(
                np.asarray(Wk)[:, cs].reshape(KC, P, CW).transpose(1, 0, 2)
            ).astype(NPBF),
            "wv": np.ascontiguousarray(
                np.asarray(Wv)[:, cs].reshape(KC, P, CW).transpose(1, 0, 2)
            ).astype(NPBF),
            "wo": np.ascontiguousarray(
                np.asarray(Wo)[cs, :].reshape(2, P, D).transpose(1, 0, 2)
            ).astype(NPBF),
            "bq3": np.asarray(bq)[cs].reshape(2, P, 1).astype(np.float32).copy(),
            "bk3": np.asarray(bk)[cs].reshape(2, P, 1).astype(np.float32).copy(),
            "bv2": np.tile(np.asarray(bv)[cs], 2).reshape(1, 2 * CW).astype(NPBF).copy(),
            "mtri": mtri,
            "ident": ident,
        })
    return in_maps


def gather(results, bo):
    bo = np.asarray(bo, np.float32)
    outs = [np.asarray(results[c]["out_p"], np.float32) for c in range(8)]
    b0 = outs[0] + outs[1] + outs[2] + outs[3] + bo
    b1 = outs[4] + outs[5] + outs[6] + outs[7] + bo
    return np.stack([b0, b1], axis=0).astype(np.float32)


def kernel(queries, keys, values, Wq, bq, Wk, bk, Wv, bv, Wo, bo):
    from concourse.bass_utils import run_bass_kernel_spmd
    nc = get_nc()
    in_maps = make_in_maps(queries, keys, values, Wq, bq, Wk, bk, Wv, bv, Wo, bo)
    res = run_bass_kernel_spmd(nc, in_maps, list(range(8)), trace=False)
    return gather(res.results, bo)

